# revision 30
# baseline (speedup 1.0000x reference)
"""Trainium2 Bass kernel for the binary CNN (nn_Net_5772436046568).

Data-parallel over 8 NeuronCores: 32 images per core, weights replicated.

Network (per core, B=32):
  conv0 (float 3->128, 3x3 SAME) + BN -> sign      [128, 32x32]
  bconv w1 -> maxpool -> sign                       [128, 16x16]
  bconv w2 -> sign                                  [256, 16x16]
  bconv w3 -> maxpool -> sign                       [256, 8x8]
  bconv w4 -> sign                                  [512, 8x8]
  bconv w5 -> maxpool -> sign                       [512, 4x4]
  fc1 (8192->1024) + BN -> hardtanh -> sign
  fc2 (1024->10) -> log_softmax

All binarized values are in {-1,0,+1} (exact in fp8e4); conv/fc sums are
small integers (exact in fp32 PSUM), so sign() == clip(h,-1,1) exactly.
Convs are 9 shifted fp8 matmuls accumulating in PSUM, reading zero-padded
SBUF activation buffers (channels on partitions, free dim = (b, Hp, Wp)).
"""

import os
import numpy as np
import ml_dtypes

NP8 = ml_dtypes.float8_e4m3
EPS = np.float32(1e-5)
S40 = np.float32(2.0**40)

_CACHE = {}
last_exec_time_ns = None


def _build():
    import concourse.mybir as mybir
    import concourse.tile as tile
    from concourse import bacc

    dt = mybir.dt
    F32 = dt.float32
    FP8 = dt.float8e4
    AL = mybir.AluOpType
    ACT = mybir.ActivationFunctionType

    nc = bacc.Bacc("TRN2", target_bir_lowering=False, debug=False, num_devices=8)

    xcold = nc.dram_tensor("xcol", (32, 27, 1024), F32, kind="ExternalInput")
    w0d = nc.dram_tensor("w0", (27, 128), F32, kind="ExternalInput")
    bn0d = nc.dram_tensor("bn0", (128, 2), F32, kind="ExternalInput")
    w1d = nc.dram_tensor("w1", (128, 1152), FP8, kind="ExternalInput")
    w2d = nc.dram_tensor("w2", (128, 2304), FP8, kind="ExternalInput")
    w3d = nc.dram_tensor("w3", (128, 4608), FP8, kind="ExternalInput")
    w4d = nc.dram_tensor("w4", (128, 9216), FP8, kind="ExternalInput")
    w5d = nc.dram_tensor("w5", (128, 18432), FP8, kind="ExternalInput")
    fc1d = nc.dram_tensor("fc1w", (64, 128, 1024), FP8, kind="ExternalInput")
    bnfd = nc.dram_tensor("bnf", (128, 16), F32, kind="ExternalInput")
    fc2d = nc.dram_tensor("fc2w", (128, 80), FP8, kind="ExternalInput")
    outd = nc.dram_tensor("out", (32, 10), F32, kind="ExternalOutput")

    with tile.TileContext(nc) as tc:
        with (
            tc.tile_pool(name="persist", bufs=1) as PP,
            tc.tile_pool(name="tmp", bufs=3) as TP,
        ):
            # ---- persistent weight tiles ----
            w0t = PP.tile([27, 128], F32)
            nc.sync.dma_start(w0t[:], w0d.ap())
            bn0t = PP.tile([128, 2], F32)
            nc.sync.dma_start(bn0t[:], bn0d.ap())
            w1t = PP.tile([128, 1152], FP8)
            nc.sync.dma_start(w1t[:], w1d.ap())
            # tiles for later layers (DMAs emitted after phase A)
            bnft = PP.tile([128, 16], F32)
            fc2t = PP.tile([128, 80], FP8)
            w2t = PP.tile([128, 2304], FP8)
            w3t = PP.tile([128, 2 * 2304], FP8)
            w4t = PP.tile([128, 2 * 4608], FP8)
            w5t = PP.tile([128, 4 * 4608], FP8)

            # ---- persistent activation buffers (padded, fp8) ----
            a2 = PP.tile([128, 32 * 18 * 32], FP8)
            a3 = PP.tile([128, 2 * 32 * 18 * 18], FP8)
            a4 = PP.tile([128, 2 * 32 * 10 * 10], FP8)
            a5 = PP.tile([128, 4 * 32 * 10 * 10], FP8)
            a6 = PP.tile([128, 4 * 32 * 16], FP8)
            a7 = PP.tile([128, 8 * 32], FP8)

            a2v = a2[:].rearrange("c (b h w) -> c b h w", b=32, h=18, w=32)
            a3v = [
                a3[:, kb * 10368 : (kb + 1) * 10368].rearrange(
                    "c (b h w) -> c b h w", b=32, h=18, w=18
                )
                for kb in range(2)
            ]
            a4v = [
                a4[:, kb * 3200 : (kb + 1) * 3200].rearrange(
                    "c (b h w) -> c b h w", b=32, h=10, w=10
                )
                for kb in range(2)
            ]
            a5v = [
                a5[:, kb * 3200 : (kb + 1) * 3200].rearrange(
                    "c (b h w) -> c b h w", b=32, h=10, w=10
                )
                for kb in range(4)
            ]

            # zero padded buffers (borders must be 0 = sign-padding); a1/a2
            # are needed early, the rest is zeroed at phase B start.
            nc.gpsimd.memzero(a2[:, : 16 * 18 * 32])
            nc.gpsimd.memzero(a2[:, 16 * 18 * 32 :])

            # PE warmup: ~8us of dummy matmuls trips the HAM un-throttle
            # (K=4/8 -> 8/8) before the fp32 conv0 stream begins.
            warm = PP.tile([128, 512], FP8)
            nc.vector.memzero(warm[:])

            def clip_store(dst, src):
                nc.vector.tensor_scalar(dst, src, -1.0, 1.0, AL.max, AL.min)

            # ============ phase A: conv0 + conv1 (a1 lives here) ============
            with (
                tc.tile_pool(name="pA", bufs=1) as PA,
                tc.tile_pool(name="xcp", bufs=3) as XC,
                tc.tile_pool(name="psA", bufs=7, space="PSUM") as PSA,
            ):
                a1 = PA.tile([128, 32 * 34 * 48], FP8)
                a1v = a1[:].rearrange("c (b h w) -> c b h w", b=32, h=34, w=48)
                Q = 8 * 34 * 48
                for qi in range(4):
                    nc.gpsimd.memzero(a1[:, qi * Q : (qi + 1) * Q])

                wps = PSA.tile([128, 512], F32, tag="warmps", bufs=1)
                for wi in range(24):
                    nc.tensor.matmul(
                        wps[:], warm[:, :128], warm[:], start=(wi == 0),
                        stop=(wi == 23),
                    )

                # conv0 (fp32, K=27 host im2col) interleaved per-image with
                # conv1 (fp8): the fp8 bursts keep the HAM clock-gate warm --
                # a pure fp32 stream does not register as PE activity and
                # would run the whole phase at 1.2 GHz.
                # conv0 chunks: one row-group = 2 output rows across a
                # 16-image half (host xcol is ordered [half*16+yg, 27, 1024])
                def conv0_grp(half, yg):
                    b0 = 16 * half
                    xc = XC.tile([27, 1024], F32)
                    nc.sync.dma_start(xc[:], xcold.ap()[half * 16 + yg])
                    for q in range(2):
                        ps = PSA.tile([128, 512], F32)
                        nc.tensor.matmul(
                            ps[:], w0t[:], xc[:, q * 512 : (q + 1) * 512],
                            start=True, stop=True,
                        )
                        t1 = TP.tile([128, 512], F32)
                        # t = (h * inv*2^40) + bias*2^40 on ACT; clip == sign
                        nc.scalar.activation(
                            t1[:], ps[:], ACT.Identity,
                            bias=bn0t[:, 1:2], scale=bn0t[:, 0:1],
                        )
                        clip_store(
                            a1v[:, b0 : b0 + 16, 1 + 2 * yg + q, 1:33], t1[:]
                        )

                # conv1 via dy-pair DoubleRow: pairs shifts (dy=0,dy=1)
                # (a1 row stride 48B is 16-aligned), dy=2 as a normal matmul.
                # Chunk = one output row y over a 16-image half (N=512).
                w1pv = w1t[:].rearrange("c (dy dx n) -> c dy dx n", dy=3, dx=3, n=128)
                DR1 = mybir.MatmulPerfMode.DoubleRow

                def conv1_rows(yp, half):
                    b0 = 16 * half
                    pss = []
                    for par in range(2):
                        y = 2 * yp + par
                        ps_ = PSA.tile(
                            [128, 512], F32, tag="ps", name=f"c1p{yp}{half}{par}"
                        )
                        pss.append(ps_)
                        i = 0
                        for dx in range(3):
                            rhs2 = a1v[:, b0 : b0 + 16, y : y + 2, dx : dx + 32]
                            nc.tensor.matmul(
                                ps_[:],
                                w1pv[:, 0:2, dx, :],
                                rhs2.transpose([0, 2, 1, 3]),
                                start=(i == 0), stop=False,
                                perf_mode=DR1, skip_group_check=True,
                            )
                            i += 1
                        for dx in range(3):
                            nc.tensor.matmul(
                                ps_[:],
                                w1pv[:, 2, dx, :],
                                a1v[:, b0 : b0 + 16, y + 2, dx : dx + 32],
                                start=False, stop=(dx == 2),
                                skip_group_check=True,
                            )
                    te = TP.tile([128, 256], F32)
                    to = TP.tile([128, 256], F32)
                    for ps_, t_ in ((pss[0], te), (pss[1], to)):
                        pvv = ps_[:].rearrange("c (b xp t) -> c b xp t", b=16, xp=16, t=2)
                        hx = TP.tile([128, 256], F32)
                        nc.scalar.copy(hx[:], pvv[:, :, :, 0])
                        nc.vector.tensor_tensor(t_[:], hx[:], pvv[:, :, :, 1], AL.max)
                    p2 = TP.tile([128, 256], F32)
                    nc.vector.tensor_tensor(p2[:], te[:], to[:], AL.max)
                    clip_store(a2v[:, b0 : b0 + 16, 1 + yp, 1:17], p2[:])

                # fine-grained fp32/fp8 interleave: conv1_rows(yp) needs
                # conv0 row-groups 0..yp+1 of its half
                wfill = [0]

                def warm_fill(n):
                    wp = PSA.tile([128, 512], F32, tag="warmps", bufs=1,
                                  name=f"wf{wfill[0]}")
                    wfill[0] += 1
                    for wi in range(n):
                        nc.tensor.matmul(
                            wp[:], warm[:, :128], warm[:],
                            start=(wi == 0), stop=(wi == n - 1),
                        )

                for half in range(2):
                    for yg in range(16):
                        conv0_grp(half, yg)
                        if yg >= 2:
                            conv1_rows(yg - 2, half)
                        else:
                            warm_fill(6)
                    conv1_rows(14, half)
                    conv1_rows(15, half)

            # ============ phase B: conv2..5, fc, softmax ============
            with tc.tile_pool(name="pB", bufs=1) as PB:
                # late weight DMAs + buffer zeroing (overlap with phase A)
                nc.sync.dma_start(w2t[:], w2d.ap())
                nc.sync.dma_start(w3t[:], w3d.ap())
                nc.sync.dma_start(w4t[:], w4d.ap())
                nc.sync.dma_start(w5t[:], w5d.ap())
                nc.sync.dma_start(bnft[:], bnfd.ap())
                nc.sync.dma_start(fc2t[:], fc2d.ap())
                for t in (a3, a4, a5):
                    nc.gpsimd.memzero(t[:])
                fc1t = PB.tile([128, 65536], FP8)
                nc.sync.dma_start(
                    fc1t[:].rearrange("c (a j) -> c a j", a=64),
                    fc1d.ap().rearrange("a c j -> c a j"),
                )

                with tc.tile_pool(name="psB", bufs=8, space="PSUM") as PSB:
                    # conv2: 128->256, 16x16, no pool -> a3 (dy-pair DR)
                    w2pv = w2t[:].rearrange(
                        "c (dy dx n) -> c dy dx n", dy=3, dx=3, n=256
                    )
                    for y in range(16):
                        for mb in range(2):
                            ps = PSB.tile([128, 512], F32, tag="ps", name=f"c2{y}{mb}")
                            i = 0
                            for dx in range(3):
                                rhs2 = a2v[:, :, y : y + 2, dx : dx + 16]
                                nc.tensor.matmul(
                                    ps[:],
                                    w2pv[:, 0:2, dx, mb * 128 : mb * 128 + 128],
                                    rhs2.transpose([0, 2, 1, 3]),
                                    start=(i == 0), stop=False,
                                    perf_mode=mybir.MatmulPerfMode.DoubleRow,
                                    skip_group_check=True,
                                )
                                i += 1
                            for dx in range(3):
                                nc.tensor.matmul(
                                    ps[:],
                                    w2pv[:, 2, dx, mb * 128 : mb * 128 + 128],
                                    a2v[:, :, y + 2, dx : dx + 16],
                                    start=False, stop=(dx == 2),
                                    skip_group_check=True,
                                )
                            clip_store(a3v[mb][:, :, 1 + y, 1:17], ps[:])

                    # conv3: 256->256, 16x16, maxpool -> a4. DoubleRow fp8:
                    # K=256 as channel-block pairs; one output row per psum
                    # (N = 32 images x 16 cols = 512), pool pairs two psums.
                    a3pv = a3[:].rearrange(
                        "c (k b h w) -> c k b h w", k=2, b=32, h=18, w=18
                    )
                    w3pv = w3t[:].rearrange(
                        "c (s mb m two) -> c s mb m two", s=9, mb=2, m=128, two=2
                    )
                    DR = mybir.MatmulPerfMode.DoubleRowSwInterleave
                    for yp in range(8):
                        for mb in range(2):
                            pse = PSB.tile([128, 512], F32, tag="ps", name=f"c3e{yp}{mb}")
                            pso = PSB.tile([128, 512], F32, tag="ps", name=f"c3o{yp}{mb}")
                            for par, ps_ in ((0, pse), (1, pso)):
                                y = 2 * yp + par
                                for s in range(9):
                                    dy, dx = divmod(s, 3)
                                    nc.tensor.matmul(
                                        ps_[:],
                                        w3pv[:, s, mb].transpose([0, 2, 1]),
                                        a3pv[:, :, :, y + dy, dx : dx + 16],
                                        start=(s == 0), stop=(s == 8),
                                        perf_mode=DR,
                                    )
                            te = TP.tile([128, 256], F32)
                            to = TP.tile([128, 256], F32)
                            for ps_, t_ in ((pse, te), (pso, to)):
                                pvv = ps_[:].rearrange(
                                    "c (b xp t) -> c b xp t", b=32, xp=8, t=2
                                )
                                hx = TP.tile([128, 256], F32)
                                nc.scalar.copy(hx[:], pvv[:, :, :, 0])
                                nc.vector.tensor_tensor(
                                    t_[:], hx[:], pvv[:, :, :, 1], AL.max
                                )
                            p2 = TP.tile([128, 256], F32)
                            nc.vector.tensor_tensor(p2[:], te[:], to[:], AL.max)
                            clip_store(a4v[mb][:, :, 1 + yp, 1:9], p2[:])

                    # conv4/conv5 DoubleRow: chunks of 64 consecutive padded
                    # rows R = b*10 + ypad across images (N = 64*8 = 512);
                    # rows with ypad in {8,9} compute garbage that the drain
                    # skips. Row counts clamp at the array end (the clipped
                    # tail rows are garbage rows anyway).
                    def img_segments(R0):
                        # [(b, y0, y1, r_off)] valid interior rows per image
                        segs = []
                        for b in range(R0 // 10, min(32, (R0 + 64 + 9) // 10)):
                            y0 = max(0, R0 - 10 * b)
                            y1 = min(8, R0 + 64 - 10 * b)
                            if y1 > y0:
                                segs.append((b, y0, y1, 10 * b + y0 - R0))
                        return segs

                    # conv4: 256->512, 8x8, no pool -> a5
                    a4pv = a4[:].rearrange("c (k r x) -> c k r x", k=2, r=320, x=10)
                    w4pv = w4t[:].rearrange(
                        "c (s mb m two) -> c s mb m two", s=9, mb=4, m=128, two=2
                    )
                    for ci in range(5):
                        R0 = 64 * ci
                        for mb in range(4):
                            ps = PSB.tile([128, 512], F32, tag="ps", name=f"c4{ci}{mb}")
                            for s in range(9):
                                dy, dx = divmod(s, 3)
                                cnt = min(64, 320 - (R0 + dy))
                                nc.tensor.matmul(
                                    ps[:, : cnt * 8],
                                    w4pv[:, s, mb].transpose([0, 2, 1]),
                                    a4pv[:, :, R0 + dy : R0 + dy + cnt, dx : dx + 8],
                                    start=(s == 0), stop=(s == 8),
                                    perf_mode=DR, skip_group_check=True,
                                )
                            psr = ps[:].rearrange("c (r x) -> c r x", x=8)
                            for b, y0, y1, ro in img_segments(R0):
                                clip_store(
                                    a5v[mb][:, b, 1 + y0 : 1 + y1, 1:9],
                                    psr[:, ro : ro + (y1 - y0), :],
                                )

                    # conv5: 512->512, 8x8, maxpool -> a6 [c, (kb b s)]
                    a5pv = a5[:].rearrange(
                        "c (g k r x) -> c g k r x", g=2, k=2, r=320, x=10
                    )
                    w5pv = w5t[:].rearrange(
                        "c (s mb g m two) -> c s mb g m two", s=9, mb=4, g=2, m=128, two=2
                    )
                    for ci in range(5):
                        R0 = 64 * ci
                        for mb in range(4):
                            ps = PSB.tile([128, 512], F32, tag="ps", name=f"c5{ci}{mb}")
                            i = 0
                            for s in range(9):
                                dy, dx = divmod(s, 3)
                                cnt = min(64, 320 - (R0 + dy))
                                for g in range(2):
                                    nc.tensor.matmul(
                                        ps[:, : cnt * 8],
                                        w5pv[:, s, mb, g].transpose([0, 2, 1]),
                                        a5pv[:, g, :, R0 + dy : R0 + dy + cnt, dx : dx + 8],
                                        start=(i == 0), stop=(i == 17),
                                        perf_mode=DR, skip_group_check=True,
                                    )
                                    i += 1
                            # xmax over col pairs, ymax over row pairs, then
                            # per-image writes of valid pooled rows
                            pvv = ps[:].rearrange("c (r xp t) -> c r xp t", r=64, xp=4, t=2)
                            hx = TP.tile([128, 256], F32)
                            nc.scalar.copy(hx[:], pvv[:, :, :, 0])
                            p1 = TP.tile([128, 256], F32)
                            nc.vector.tensor_tensor(p1[:], hx[:], pvv[:, :, :, 1], AL.max)
                            p1v = p1[:].rearrange("c (r2 t xp) -> c r2 t xp", r2=32, t=2, xp=4)
                            p2 = TP.tile([128, 128], F32)
                            nc.vector.tensor_tensor(
                                p2[:], p1v[:, :, 0, :], p1v[:, :, 1, :], AL.max
                            )
                            # p2 cols: (R0/2 + r2) -> image b = (R0+2*r2)//10,
                            # pooled row m = ((R0+2*r2)%10)//2 valid if %10 < 8
                            for b in range(R0 // 10, min(32, (R0 + 64 + 9) // 10)):
                                m0 = max(0, -(-(R0 - 10 * b) // 2))
                                m1 = min(4, (R0 + 64 - 10 * b) // 2)
                                if m1 > m0:
                                    c0 = (10 * b + 2 * m0 - R0) // 2 * 4
                                    clip_store(
                                        a6[:, mb * 512 + b * 16 + m0 * 4 : mb * 512 + b * 16 + m1 * 4],
                                        p2[:, c0 : c0 + (m1 - m0) * 4],
                                    )

                # fc1: [32, 8192] @ [8192, 1024], j on partitions
                a6v = a6[:].rearrange("c (k b s) -> c k b s", k=4, b=32, s=16)
                with tc.tile_pool(name="psF", bufs=8, space="PSUM") as PSF:
                    psF = [PSF.tile([128, 32], F32, tag="psF", name=f"psF{j}") for j in range(8)]
                    for si in range(16):
                        for kb in range(4):
                            rhs = a6v[:, kb, :, si]
                            for jb in range(8):
                                c0 = (si * 4 + kb) * 1024 + jb * 128
                                nc.tensor.matmul(
                                    psF[jb][:],
                                    fc1t[:, c0 : c0 + 128],
                                    rhs,
                                    start=(si == 0 and kb == 0),
                                    stop=(si == 15 and kb == 3),
                                    skip_group_check=True,
                                )
                    for jb in range(8):
                        t1 = TP.tile([128, 32], F32)
                        nc.vector.tensor_scalar(
                            t1[:], psF[jb][:],
                            bnft[:, jb : jb + 1], bnft[:, 8 + jb : 9 + jb],
                            AL.mult, AL.add,
                        )
                        clip_store(a7[:, jb * 32 : (jb + 1) * 32], t1[:])

                # fc2 + log_softmax
                with tc.tile_pool(name="psZ", bufs=2, space="PSUM") as PSZ:
                    zps = PSZ.tile([32, 10], F32)
                    for jb in range(8):
                        nc.tensor.matmul(
                            zps[:],
                            a7[:, jb * 32 : (jb + 1) * 32],
                            fc2t[:, jb * 10 : (jb + 1) * 10],
                            start=(jb == 0), stop=(jb == 7),
                        )
                    mx = TP.tile([32, 1], F32)
                    nc.vector.pool_max(mx[:], zps[:])
                    ng = TP.tile([32, 1], F32)
                    nc.vector.tensor_scalar(ng[:], mx[:], -1.0, None, AL.mult)
                    ex = TP.tile([32, 10], F32)
                    sm = TP.tile([32, 1], F32)
                    nc.scalar.activation(
                        ex[:], zps[:], ACT.Exp, bias=ng[:], scale=1.0, accum_out=sm[:]
                    )
                    ls = TP.tile([32, 1], F32)
                    nc.scalar.activation(ls[:], sm[:], ACT.Ln)
                    nls = TP.tile([32, 1], F32)
                    nc.vector.tensor_scalar(nls[:], ls[:], -1.0, None, AL.mult)
                    ot = TP.tile([32, 10], F32)
                    nc.vector.tensor_scalar(ot[:], zps[:], ng[:], nls[:], AL.add, AL.add)
                    nc.sync.dma_start(outd.ap(), ot[:])

    nc.finalize()
    return nc


def _get_nc():
    if "nc" not in _CACHE:
        _CACHE["nc"] = _build()
    return _CACHE["nc"]


def _prep_inputs(x, w0, bn1_g, bn1_b, bn1_m, bn1_v, w1, w2, w3, w4, w5,
                 fc1_w, bnf_g, bnf_b, bnf_m, bnf_v, fc2_w):
    f32 = np.float32
    x = np.asarray(x, f32)

    inv1 = (np.asarray(bn1_g, f32) / np.sqrt(np.asarray(bn1_v, f32) + EPS)).astype(f32)
    bias1 = (np.asarray(bn1_b, f32) - np.asarray(bn1_m, f32) * inv1).astype(f32)
    bn0_h = np.stack([inv1 * S40, bias1 * S40], axis=1).astype(f32)  # [128,2]

    invf = (np.asarray(bnf_g, f32) / np.sqrt(np.asarray(bnf_v, f32) + EPS)).astype(f32)
    biasf = (np.asarray(bnf_b, f32) - np.asarray(bnf_m, f32) * invf).astype(f32)
    bnf_h = np.concatenate(
        [(invf * S40).reshape(8, 128).T, (biasf * S40).reshape(8, 128).T], axis=1
    ).astype(f32)  # [128, 16] cols: inv per jb, then bias per jb

    w0_h = np.ascontiguousarray(
        np.asarray(w0, f32).transpose(2, 3, 1, 0).reshape(27, 128)
    )

    def conv_w(w):  # [Cout, Cin, 3, 3] -> sign -> [Cin, 9*Cout] fp8
        w = np.asarray(w, f32)
        cout, cin = w.shape[0], w.shape[1]
        return np.ascontiguousarray(
            np.sign(w).transpose(1, 2, 3, 0).reshape(cin, 9 * cout)
        ).astype(NP8)

    w1_h, w2_h = map(conv_w, (w1, w2))

    def conv_w_swi(w, mbn, gn):
        # [Cout, Cin, 3, 3] -> [128ci, (s, mb, (g,) co_rev, j)] fp8,
        # pairs (A,B)=(kb 2g, 2g+1) interleaved per column, columns reversed
        w = np.sign(np.asarray(w, np.float32))
        t = w.reshape(mbn, 128, gn, 2, 128, 3, 3)  # [mb, co, g, j, ci, dy, dx]
        t = t.transpose(4, 5, 6, 0, 2, 1, 3)       # [ci, dy, dx, mb, g, co, j]
        t = t[:, :, :, :, :, ::-1, :]              # co reversed
        return np.ascontiguousarray(t.reshape(128, -1)).astype(NP8)

    w3_h = conv_w_swi(w3, 2, 1)
    w4_h = conv_w_swi(w4, 4, 1)
    w5_h = conv_w_swi(w5, 4, 2)

    fc1_h = np.ascontiguousarray(
        np.sign(np.asarray(fc1_w, f32))
        .reshape(1024, 512, 16)
        .transpose(2, 1, 0)
        .reshape(64, 128, 1024)
    ).astype(NP8)

    fc2_h = np.ascontiguousarray(
        np.sign(np.asarray(fc2_w, f32)).T.reshape(8, 128, 10).transpose(1, 0, 2).reshape(128, 80)
    ).astype(NP8)

    common = {
        "w0": w0_h, "bn0": bn0_h, "w1": w1_h, "w2": w2_h, "w3": w3_h,
        "w4": w4_h, "w5": w5_h, "fc1w": fc1_h, "bnf": bnf_h, "fc2w": fc2_h,
    }
    in_maps = []
    for c in range(8):
        xs = x[c * 32 : (c + 1) * 32]
        xp = np.zeros((32, 3, 34, 34), f32)
        xp[:, :, 1:33, 1:33] = xs
        # im2col rows (s,ci): [16 chunks, 27, 2 images * 32 * 32]
        win = np.stack(
            [xp[:, :, dy : dy + 32, dx : dx + 32]
             for dy in range(3) for dx in range(3)],
            axis=1,
        )  # [32, 9, 3, 32, 32]
        # reorder to row-groups: [half*16+yg, 27, (ysub2, b16, x32)]
        xcol = np.ascontiguousarray(
            win.reshape(2, 16, 9, 3, 32, 32)   # [half, b, s, ci, y, x]
            .transpose(0, 4, 2, 3, 1, 5)       # [half, y, s, ci, b, x]
            .reshape(2, 16, 2, 27, 16, 32)     # [half, yg, ysub, sci, b, x]
            .transpose(0, 1, 3, 2, 4, 5)       # [half, yg, sci, ysub, b, x]
            .reshape(32, 27, 1024)
        )
        in_maps.append({**common, "xcol": xcol})
    return in_maps


def _install_trace_shim():
    """The image's antenv lacks axon_hooks; recreate it from trn_agent_boot
    so run_bass_kernel_spmd(trace=True) can capture NTFF profiles."""
    import sys, types
    if "antenv.axon_hooks" in sys.modules:
        return True
    try:
        from trn_agent_boot.trn_boot import _ntff_profile_via_ctypes
        h = _ntff_profile_via_ctypes("/opt/axon/libaxon_pjrt.so")
        if h is None:
            return False
        m = types.ModuleType("antenv.axon_hooks")
        m.get_axon_ntff_profile_hook = lambda: h
        m.set_axon_ntff_profile_hook = lambda hook: None
        sys.modules["antenv.axon_hooks"] = m
        import antenv
        antenv.axon_hooks = m
        return True
    except Exception:
        return False


def kernel(**inputs):
    global last_exec_time_ns
    from concourse import bass_utils

    nc = _get_nc()
    in_maps = _prep_inputs(**inputs)
    trace = bool(int(os.environ.get("BASS_KERNEL_TRACE", "0")))
    if trace:
        trace = _install_trace_shim()
    res = bass_utils.run_bass_kernel_spmd(
        nc, in_maps, core_ids=list(range(8)), trace=trace
    )
    last_exec_time_ns = res.exec_time_ns
    _CACHE["last_results"] = res
    return np.concatenate(
        [res.results[c]["out"] for c in range(8)], axis=0
    ).astype(np.float32)


# revision 31
# speedup vs baseline: 1.2033x; 1.2033x over previous
"""Trainium2 Bass kernel for the binary CNN (nn_Net_5772436046568).

Data-parallel over 8 NeuronCores: 32 images per core, weights replicated.

Network (per core, B=32):
  conv0 (float 3->128, 3x3 SAME) + BN -> sign      [128, 32x32]
  bconv w1 -> maxpool -> sign                       [128, 16x16]
  bconv w2 -> sign                                  [256, 16x16]
  bconv w3 -> maxpool -> sign                       [256, 8x8]
  bconv w4 -> sign                                  [512, 8x8]
  bconv w5 -> maxpool -> sign                       [512, 4x4]
  fc1 (8192->1024) + BN -> hardtanh -> sign
  fc2 (1024->10) -> log_softmax

All binarized values are in {-1,0,+1} (exact in fp8e4); conv/fc sums are
small integers (exact in fp32 PSUM), so sign() == clip(h,-1,1) exactly.
Convs are 9 shifted fp8 matmuls accumulating in PSUM, reading zero-padded
SBUF activation buffers (channels on partitions, free dim = (b, Hp, Wp)).
"""

import os
import numpy as np
import ml_dtypes

NP8 = ml_dtypes.float8_e4m3
EPS = np.float32(1e-5)
S40 = np.float32(2.0**40)

_CACHE = {}
last_exec_time_ns = None


def _build():
    import concourse.mybir as mybir
    import concourse.tile as tile
    from concourse import bacc

    dt = mybir.dt
    F32 = dt.float32
    FP8 = dt.float8e4
    AL = mybir.AluOpType
    ACT = mybir.ActivationFunctionType

    nc = bacc.Bacc("TRN2", target_bir_lowering=False, debug=False, num_devices=8)

    xcold = nc.dram_tensor("xcol", (32, 27, 1024), F32, kind="ExternalInput")
    w0d = nc.dram_tensor("w0", (27, 128), F32, kind="ExternalInput")
    bn0d = nc.dram_tensor("bn0", (128, 2), F32, kind="ExternalInput")
    w1d = nc.dram_tensor("w1", (128, 1152), FP8, kind="ExternalInput")
    w2d = nc.dram_tensor("w2", (128, 2304), FP8, kind="ExternalInput")
    w3d = nc.dram_tensor("w3", (128, 4608), FP8, kind="ExternalInput")
    w4d = nc.dram_tensor("w4", (128, 9216), FP8, kind="ExternalInput")
    w5d = nc.dram_tensor("w5", (128, 18432), FP8, kind="ExternalInput")
    fc1d = nc.dram_tensor("fc1w", (64, 128, 1024), FP8, kind="ExternalInput")
    bnfd = nc.dram_tensor("bnf", (128, 16), F32, kind="ExternalInput")
    fc2d = nc.dram_tensor("fc2w", (128, 80), FP8, kind="ExternalInput")
    outd = nc.dram_tensor("out", (32, 10), F32, kind="ExternalOutput")

    with tile.TileContext(nc) as tc:
        with (
            tc.tile_pool(name="persist", bufs=1) as PP,
            tc.tile_pool(name="tmp", bufs=3) as TP,
        ):
            # ---- persistent weight tiles ----
            w0t = PP.tile([27, 128], F32)
            nc.sync.dma_start(w0t[:], w0d.ap())
            bn0t = PP.tile([128, 2], F32)
            nc.sync.dma_start(bn0t[:], bn0d.ap())
            w1t = PP.tile([128, 1152], FP8)
            nc.sync.dma_start(w1t[:], w1d.ap())
            # tiles for later layers (DMAs emitted after phase A)
            bnft = PP.tile([128, 16], F32)
            fc2t = PP.tile([128, 80], FP8)
            w2t = PP.tile([128, 2304], FP8)
            w3t = PP.tile([128, 2 * 2304], FP8)
            w4t = PP.tile([128, 2 * 4608], FP8)
            w5t = PP.tile([128, 4 * 4608], FP8)

            # ---- persistent activation buffers (padded, fp8) ----
            a2 = PP.tile([128, 32 * 18 * 32], FP8)
            a3 = PP.tile([128, 2 * 32 * 18 * 18], FP8)
            a4 = PP.tile([128, 2 * 32 * 10 * 10], FP8)
            a5 = PP.tile([128, 4 * 32 * 10 * 10], FP8)
            a6 = PP.tile([128, 4 * 32 * 16], FP8)
            a7 = PP.tile([128, 8 * 32], FP8)

            a2v = a2[:].rearrange("c (b h w) -> c b h w", b=32, h=18, w=32)
            a3v = [
                a3[:, kb * 10368 : (kb + 1) * 10368].rearrange(
                    "c (b h w) -> c b h w", b=32, h=18, w=18
                )
                for kb in range(2)
            ]
            a4v = [
                a4[:, kb * 3200 : (kb + 1) * 3200].rearrange(
                    "c (b h w) -> c b h w", b=32, h=10, w=10
                )
                for kb in range(2)
            ]
            a5v = [
                a5[:, kb * 3200 : (kb + 1) * 3200].rearrange(
                    "c (b h w) -> c b h w", b=32, h=10, w=10
                )
                for kb in range(4)
            ]

            # zero padded buffers (borders must be 0 = sign-padding); a1/a2
            # are needed early, the rest is zeroed at phase B start.
            nc.gpsimd.memzero(a2[:, : 16 * 18 * 32])
            nc.gpsimd.memzero(a2[:, 16 * 18 * 32 :])

            # PE warmup: ~8us of dummy matmuls trips the HAM un-throttle
            # (K=4/8 -> 8/8) before the fp32 conv0 stream begins.
            warm = PP.tile([128, 512], FP8)
            nc.vector.memzero(warm[:])

            def clip_store(dst, src):
                nc.vector.tensor_scalar(dst, src, -1.0, 1.0, AL.max, AL.min)

            # ============ phase A: conv0 + conv1 (a1 lives here) ============
            with (
                tc.tile_pool(name="pA", bufs=1) as PA,
                tc.tile_pool(name="xcp", bufs=3) as XC,
                tc.tile_pool(name="psA", bufs=7, space="PSUM") as PSA,
            ):
                a1 = PA.tile([128, 32 * 34 * 48], FP8)
                a1v = a1[:].rearrange("c (b h w) -> c b h w", b=32, h=34, w=48)
                Q = 8 * 34 * 48
                for qi in range(4):
                    nc.gpsimd.memzero(a1[:, qi * Q : (qi + 1) * Q])

                wps = PSA.tile([128, 512], F32, tag="warmps", bufs=1)
                for wi in range(24):
                    nc.tensor.matmul(
                        wps[:], warm[:, :128], warm[:], start=(wi == 0),
                        stop=(wi == 23),
                    )

                # conv0 (fp32, K=27 host im2col) interleaved per-image with
                # conv1 (fp8): the fp8 bursts keep the HAM clock-gate warm --
                # a pure fp32 stream does not register as PE activity and
                # would run the whole phase at 1.2 GHz.
                # conv0 chunks: one row-group = 2 output rows across a
                # 16-image half (host xcol is ordered [half*16+yg, 27, 1024])
                def conv0_grp(half, yg):
                    b0 = 16 * half
                    xc = XC.tile([27, 1024], F32)
                    nc.sync.dma_start(xc[:], xcold.ap()[half * 16 + yg])
                    for q in range(2):
                        ps = PSA.tile([128, 512], F32)
                        nc.tensor.matmul(
                            ps[:], w0t[:], xc[:, q * 512 : (q + 1) * 512],
                            start=True, stop=True,
                        )
                        t1 = TP.tile([128, 512], F32)
                        # t = (h * inv*2^40) + bias*2^40 on ACT; clip == sign
                        nc.scalar.activation(
                            t1[:], ps[:], ACT.Identity,
                            bias=bn0t[:, 1:2], scale=bn0t[:, 0:1],
                        )
                        clip_store(
                            a1v[:, b0 : b0 + 16, 1 + 2 * yg + q, 1:33], t1[:]
                        )

                # conv1 via dy-pair DoubleRow: pairs shifts (dy=0,dy=1)
                # (a1 row stride 48B is 16-aligned), dy=2 as a normal matmul.
                # Chunk = one output row y over a 16-image half (N=512).
                w1pv = w1t[:].rearrange("c (dy dx n) -> c dy dx n", dy=3, dx=3, n=128)
                DR1 = mybir.MatmulPerfMode.DoubleRow

                def conv1_rows(yp, half):
                    b0 = 16 * half
                    pss = []
                    for par in range(2):
                        y = 2 * yp + par
                        ps_ = PSA.tile(
                            [128, 512], F32, tag="ps", name=f"c1p{yp}{half}{par}"
                        )
                        pss.append(ps_)
                        i = 0
                        for dx in range(3):
                            rhs2 = a1v[:, b0 : b0 + 16, y : y + 2, dx : dx + 32]
                            nc.tensor.matmul(
                                ps_[:],
                                w1pv[:, 0:2, dx, :],
                                rhs2.transpose([0, 2, 1, 3]),
                                start=(i == 0), stop=False,
                                perf_mode=DR1, skip_group_check=True,
                            )
                            i += 1
                        for dx in range(3):
                            nc.tensor.matmul(
                                ps_[:],
                                w1pv[:, 2, dx, :],
                                a1v[:, b0 : b0 + 16, y + 2, dx : dx + 32],
                                start=False, stop=(dx == 2),
                                skip_group_check=True,
                            )
                    te = TP.tile([128, 256], F32)
                    to = TP.tile([128, 256], F32)
                    for ps_, t_ in ((pss[0], te), (pss[1], to)):
                        pvv = ps_[:].rearrange("c (b xp t) -> c b xp t", b=16, xp=16, t=2)
                        hx = TP.tile([128, 256], F32)
                        nc.scalar.copy(hx[:], pvv[:, :, :, 0])
                        nc.vector.tensor_tensor(t_[:], hx[:], pvv[:, :, :, 1], AL.max)
                    p2 = TP.tile([128, 256], F32)
                    nc.vector.tensor_tensor(p2[:], te[:], to[:], AL.max)
                    clip_store(a2v[:, b0 : b0 + 16, 1 + yp, 1:17], p2[:])

                # fine-grained fp32/fp8 interleave: conv1_rows(yp) needs
                # conv0 row-groups 0..yp+1 of its half
                for half in range(2):
                    for yg in range(16):
                        conv0_grp(half, yg)
                        if yg >= 2:
                            conv1_rows(yg - 2, half)
                    conv1_rows(14, half)
                    conv1_rows(15, half)

            # ============ phase B: conv2..5, fc, softmax ============
            with tc.tile_pool(name="pB", bufs=1) as PB:
                # late weight DMAs + buffer zeroing (overlap with phase A)
                nc.sync.dma_start(w2t[:], w2d.ap())
                nc.sync.dma_start(w3t[:], w3d.ap())
                nc.sync.dma_start(w4t[:], w4d.ap())
                nc.sync.dma_start(w5t[:], w5d.ap())
                nc.sync.dma_start(bnft[:], bnfd.ap())
                nc.sync.dma_start(fc2t[:], fc2d.ap())
                for t in (a3, a4, a5):
                    nc.gpsimd.memzero(t[:])
                fc1t = PB.tile([128, 65536], FP8)
                nc.sync.dma_start(
                    fc1t[:].rearrange("c (a j) -> c a j", a=64),
                    fc1d.ap().rearrange("a c j -> c a j"),
                )

                with tc.tile_pool(name="psB", bufs=8, space="PSUM") as PSB:
                    # conv2: 128->256, 16x16, no pool -> a3 (dy-pair DR)
                    w2pv = w2t[:].rearrange(
                        "c (dy dx n) -> c dy dx n", dy=3, dx=3, n=256
                    )
                    for y in range(16):
                        for mb in range(2):
                            ps = PSB.tile([128, 512], F32, tag="ps", name=f"c2{y}{mb}")
                            i = 0
                            for dx in range(3):
                                rhs2 = a2v[:, :, y : y + 2, dx : dx + 16]
                                nc.tensor.matmul(
                                    ps[:],
                                    w2pv[:, 0:2, dx, mb * 128 : mb * 128 + 128],
                                    rhs2.transpose([0, 2, 1, 3]),
                                    start=(i == 0), stop=False,
                                    perf_mode=mybir.MatmulPerfMode.DoubleRow,
                                    skip_group_check=True,
                                )
                                i += 1
                            for dx in range(3):
                                nc.tensor.matmul(
                                    ps[:],
                                    w2pv[:, 2, dx, mb * 128 : mb * 128 + 128],
                                    a2v[:, :, y + 2, dx : dx + 16],
                                    start=False, stop=(dx == 2),
                                    skip_group_check=True,
                                )
                            clip_store(a3v[mb][:, :, 1 + y, 1:17], ps[:])

                    # conv3: 256->256, 16x16, maxpool -> a4. DoubleRow fp8:
                    # K=256 as channel-block pairs; one output row per psum
                    # (N = 32 images x 16 cols = 512), pool pairs two psums.
                    a3pv = a3[:].rearrange(
                        "c (k b h w) -> c k b h w", k=2, b=32, h=18, w=18
                    )
                    w3pv = w3t[:].rearrange(
                        "c (s mb m two) -> c s mb m two", s=9, mb=2, m=128, two=2
                    )
                    DR = mybir.MatmulPerfMode.DoubleRowSwInterleave
                    for yp in range(8):
                        for mb in range(2):
                            pse = PSB.tile([128, 512], F32, tag="ps", name=f"c3e{yp}{mb}")
                            pso = PSB.tile([128, 512], F32, tag="ps", name=f"c3o{yp}{mb}")
                            for par, ps_ in ((0, pse), (1, pso)):
                                y = 2 * yp + par
                                for s in range(9):
                                    dy, dx = divmod(s, 3)
                                    nc.tensor.matmul(
                                        ps_[:],
                                        w3pv[:, s, mb].transpose([0, 2, 1]),
                                        a3pv[:, :, :, y + dy, dx : dx + 16],
                                        start=(s == 0), stop=(s == 8),
                                        perf_mode=DR,
                                    )
                            te = TP.tile([128, 256], F32)
                            to = TP.tile([128, 256], F32)
                            for ps_, t_ in ((pse, te), (pso, to)):
                                pvv = ps_[:].rearrange(
                                    "c (b xp t) -> c b xp t", b=32, xp=8, t=2
                                )
                                hx = TP.tile([128, 256], F32)
                                nc.scalar.copy(hx[:], pvv[:, :, :, 0])
                                nc.vector.tensor_tensor(
                                    t_[:], hx[:], pvv[:, :, :, 1], AL.max
                                )
                            p2 = TP.tile([128, 256], F32)
                            nc.vector.tensor_tensor(p2[:], te[:], to[:], AL.max)
                            clip_store(a4v[mb][:, :, 1 + yp, 1:9], p2[:])

                    # conv4/conv5 DoubleRow: chunks of 64 consecutive padded
                    # rows R = b*10 + ypad across images (N = 64*8 = 512);
                    # rows with ypad in {8,9} compute garbage that the drain
                    # skips. Row counts clamp at the array end (the clipped
                    # tail rows are garbage rows anyway).
                    def img_segments(R0):
                        # [(b, y0, y1, r_off)] valid interior rows per image
                        segs = []
                        for b in range(R0 // 10, min(32, (R0 + 64 + 9) // 10)):
                            y0 = max(0, R0 - 10 * b)
                            y1 = min(8, R0 + 64 - 10 * b)
                            if y1 > y0:
                                segs.append((b, y0, y1, 10 * b + y0 - R0))
                        return segs

                    # conv4: 256->512, 8x8, no pool -> a5
                    a4pv = a4[:].rearrange("c (k r x) -> c k r x", k=2, r=320, x=10)
                    w4pv = w4t[:].rearrange(
                        "c (s mb m two) -> c s mb m two", s=9, mb=4, m=128, two=2
                    )
                    for ci in range(5):
                        R0 = 64 * ci
                        for mb in range(4):
                            ps = PSB.tile([128, 512], F32, tag="ps", name=f"c4{ci}{mb}")
                            for s in range(9):
                                dy, dx = divmod(s, 3)
                                cnt = min(64, 320 - (R0 + dy))
                                nc.tensor.matmul(
                                    ps[:, : cnt * 8],
                                    w4pv[:, s, mb].transpose([0, 2, 1]),
                                    a4pv[:, :, R0 + dy : R0 + dy + cnt, dx : dx + 8],
                                    start=(s == 0), stop=(s == 8),
                                    perf_mode=DR, skip_group_check=True,
                                )
                            psr = ps[:].rearrange("c (r x) -> c r x", x=8)
                            for b, y0, y1, ro in img_segments(R0):
                                clip_store(
                                    a5v[mb][:, b, 1 + y0 : 1 + y1, 1:9],
                                    psr[:, ro : ro + (y1 - y0), :],
                                )

                    # conv5: 512->512, 8x8, maxpool -> a6 [c, (kb b s)]
                    a5pv = a5[:].rearrange(
                        "c (g k r x) -> c g k r x", g=2, k=2, r=320, x=10
                    )
                    w5pv = w5t[:].rearrange(
                        "c (s mb g m two) -> c s mb g m two", s=9, mb=4, g=2, m=128, two=2
                    )
                    for ci in range(5):
                        R0 = 64 * ci
                        for mb in range(4):
                            ps = PSB.tile([128, 512], F32, tag="ps", name=f"c5{ci}{mb}")
                            i = 0
                            for s in range(9):
                                dy, dx = divmod(s, 3)
                                cnt = min(64, 320 - (R0 + dy))
                                for g in range(2):
                                    nc.tensor.matmul(
                                        ps[:, : cnt * 8],
                                        w5pv[:, s, mb, g].transpose([0, 2, 1]),
                                        a5pv[:, g, :, R0 + dy : R0 + dy + cnt, dx : dx + 8],
                                        start=(i == 0), stop=(i == 17),
                                        perf_mode=DR, skip_group_check=True,
                                    )
                                    i += 1
                            # xmax over col pairs, ymax over row pairs, then
                            # per-image writes of valid pooled rows
                            pvv = ps[:].rearrange("c (r xp t) -> c r xp t", r=64, xp=4, t=2)
                            hx = TP.tile([128, 256], F32)
                            nc.scalar.copy(hx[:], pvv[:, :, :, 0])
                            p1 = TP.tile([128, 256], F32)
                            nc.vector.tensor_tensor(p1[:], hx[:], pvv[:, :, :, 1], AL.max)
                            p1v = p1[:].rearrange("c (r2 t xp) -> c r2 t xp", r2=32, t=2, xp=4)
                            p2 = TP.tile([128, 128], F32)
                            nc.vector.tensor_tensor(
                                p2[:], p1v[:, :, 0, :], p1v[:, :, 1, :], AL.max
                            )
                            # p2 cols: (R0/2 + r2) -> image b = (R0+2*r2)//10,
                            # pooled row m = ((R0+2*r2)%10)//2 valid if %10 < 8
                            for b in range(R0 // 10, min(32, (R0 + 64 + 9) // 10)):
                                m0 = max(0, -(-(R0 - 10 * b) // 2))
                                m1 = min(4, (R0 + 64 - 10 * b) // 2)
                                if m1 > m0:
                                    c0 = (10 * b + 2 * m0 - R0) // 2 * 4
                                    clip_store(
                                        a6[:, mb * 512 + b * 16 + m0 * 4 : mb * 512 + b * 16 + m1 * 4],
                                        p2[:, c0 : c0 + (m1 - m0) * 4],
                                    )

                # fc1: [32, 8192] @ [8192, 1024], j on partitions
                a6v = a6[:].rearrange("c (k b s) -> c k b s", k=4, b=32, s=16)
                with tc.tile_pool(name="psF", bufs=8, space="PSUM") as PSF:
                    psF = [PSF.tile([128, 32], F32, tag="psF", name=f"psF{j}") for j in range(8)]
                    for si in range(16):
                        for kb in range(4):
                            rhs = a6v[:, kb, :, si]
                            for jb in range(8):
                                c0 = (si * 4 + kb) * 1024 + jb * 128
                                nc.tensor.matmul(
                                    psF[jb][:],
                                    fc1t[:, c0 : c0 + 128],
                                    rhs,
                                    start=(si == 0 and kb == 0),
                                    stop=(si == 15 and kb == 3),
                                    skip_group_check=True,
                                )
                    for jb in range(8):
                        t1 = TP.tile([128, 32], F32)
                        nc.vector.tensor_scalar(
                            t1[:], psF[jb][:],
                            bnft[:, jb : jb + 1], bnft[:, 8 + jb : 9 + jb],
                            AL.mult, AL.add,
                        )
                        clip_store(a7[:, jb * 32 : (jb + 1) * 32], t1[:])

                # fc2 + log_softmax
                with tc.tile_pool(name="psZ", bufs=2, space="PSUM") as PSZ:
                    zps = PSZ.tile([32, 10], F32)
                    for jb in range(8):
                        nc.tensor.matmul(
                            zps[:],
                            a7[:, jb * 32 : (jb + 1) * 32],
                            fc2t[:, jb * 10 : (jb + 1) * 10],
                            start=(jb == 0), stop=(jb == 7),
                        )
                    mx = TP.tile([32, 1], F32)
                    nc.vector.pool_max(mx[:], zps[:])
                    ng = TP.tile([32, 1], F32)
                    nc.vector.tensor_scalar(ng[:], mx[:], -1.0, None, AL.mult)
                    ex = TP.tile([32, 10], F32)
                    sm = TP.tile([32, 1], F32)
                    nc.scalar.activation(
                        ex[:], zps[:], ACT.Exp, bias=ng[:], scale=1.0, accum_out=sm[:]
                    )
                    ls = TP.tile([32, 1], F32)
                    nc.scalar.activation(ls[:], sm[:], ACT.Ln)
                    nls = TP.tile([32, 1], F32)
                    nc.vector.tensor_scalar(nls[:], ls[:], -1.0, None, AL.mult)
                    ot = TP.tile([32, 10], F32)
                    nc.vector.tensor_scalar(ot[:], zps[:], ng[:], nls[:], AL.add, AL.add)
                    nc.sync.dma_start(outd.ap(), ot[:])

    nc.finalize()
    return nc


def _get_nc():
    if "nc" not in _CACHE:
        _CACHE["nc"] = _build()
    return _CACHE["nc"]


def _prep_inputs(x, w0, bn1_g, bn1_b, bn1_m, bn1_v, w1, w2, w3, w4, w5,
                 fc1_w, bnf_g, bnf_b, bnf_m, bnf_v, fc2_w):
    f32 = np.float32
    x = np.asarray(x, f32)

    inv1 = (np.asarray(bn1_g, f32) / np.sqrt(np.asarray(bn1_v, f32) + EPS)).astype(f32)
    bias1 = (np.asarray(bn1_b, f32) - np.asarray(bn1_m, f32) * inv1).astype(f32)
    bn0_h = np.stack([inv1 * S40, bias1 * S40], axis=1).astype(f32)  # [128,2]

    invf = (np.asarray(bnf_g, f32) / np.sqrt(np.asarray(bnf_v, f32) + EPS)).astype(f32)
    biasf = (np.asarray(bnf_b, f32) - np.asarray(bnf_m, f32) * invf).astype(f32)
    bnf_h = np.concatenate(
        [(invf * S40).reshape(8, 128).T, (biasf * S40).reshape(8, 128).T], axis=1
    ).astype(f32)  # [128, 16] cols: inv per jb, then bias per jb

    w0_h = np.ascontiguousarray(
        np.asarray(w0, f32).transpose(2, 3, 1, 0).reshape(27, 128)
    )

    def conv_w(w):  # [Cout, Cin, 3, 3] -> sign -> [Cin, 9*Cout] fp8
        w = np.asarray(w, f32)
        cout, cin = w.shape[0], w.shape[1]
        return np.ascontiguousarray(
            np.sign(w).transpose(1, 2, 3, 0).reshape(cin, 9 * cout)
        ).astype(NP8)

    w1_h, w2_h = map(conv_w, (w1, w2))

    def conv_w_swi(w, mbn, gn):
        # [Cout, Cin, 3, 3] -> [128ci, (s, mb, (g,) co_rev, j)] fp8,
        # pairs (A,B)=(kb 2g, 2g+1) interleaved per column, columns reversed
        w = np.sign(np.asarray(w, np.float32))
        t = w.reshape(mbn, 128, gn, 2, 128, 3, 3)  # [mb, co, g, j, ci, dy, dx]
        t = t.transpose(4, 5, 6, 0, 2, 1, 3)       # [ci, dy, dx, mb, g, co, j]
        t = t[:, :, :, :, :, ::-1, :]              # co reversed
        return np.ascontiguousarray(t.reshape(128, -1)).astype(NP8)

    w3_h = conv_w_swi(w3, 2, 1)
    w4_h = conv_w_swi(w4, 4, 1)
    w5_h = conv_w_swi(w5, 4, 2)

    fc1_h = np.ascontiguousarray(
        np.sign(np.asarray(fc1_w, f32))
        .reshape(1024, 512, 16)
        .transpose(2, 1, 0)
        .reshape(64, 128, 1024)
    ).astype(NP8)

    fc2_h = np.ascontiguousarray(
        np.sign(np.asarray(fc2_w, f32)).T.reshape(8, 128, 10).transpose(1, 0, 2).reshape(128, 80)
    ).astype(NP8)

    common = {
        "w0": w0_h, "bn0": bn0_h, "w1": w1_h, "w2": w2_h, "w3": w3_h,
        "w4": w4_h, "w5": w5_h, "fc1w": fc1_h, "bnf": bnf_h, "fc2w": fc2_h,
    }
    in_maps = []
    for c in range(8):
        xs = x[c * 32 : (c + 1) * 32]
        xp = np.zeros((32, 3, 34, 34), f32)
        xp[:, :, 1:33, 1:33] = xs
        # im2col rows (s,ci): [16 chunks, 27, 2 images * 32 * 32]
        win = np.stack(
            [xp[:, :, dy : dy + 32, dx : dx + 32]
             for dy in range(3) for dx in range(3)],
            axis=1,
        )  # [32, 9, 3, 32, 32]
        # reorder to row-groups: [half*16+yg, 27, (ysub2, b16, x32)]
        xcol = np.ascontiguousarray(
            win.reshape(2, 16, 9, 3, 32, 32)   # [half, b, s, ci, y, x]
            .transpose(0, 4, 2, 3, 1, 5)       # [half, y, s, ci, b, x]
            .reshape(2, 16, 2, 27, 16, 32)     # [half, yg, ysub, sci, b, x]
            .transpose(0, 1, 3, 2, 4, 5)       # [half, yg, sci, ysub, b, x]
            .reshape(32, 27, 1024)
        )
        in_maps.append({**common, "xcol": xcol})
    return in_maps


def _install_trace_shim():
    """The image's antenv lacks axon_hooks; recreate it from trn_agent_boot
    so run_bass_kernel_spmd(trace=True) can capture NTFF profiles."""
    import sys, types
    if "antenv.axon_hooks" in sys.modules:
        return True
    try:
        from trn_agent_boot.trn_boot import _ntff_profile_via_ctypes
        h = _ntff_profile_via_ctypes("/opt/axon/libaxon_pjrt.so")
        if h is None:
            return False
        m = types.ModuleType("antenv.axon_hooks")
        m.get_axon_ntff_profile_hook = lambda: h
        m.set_axon_ntff_profile_hook = lambda hook: None
        sys.modules["antenv.axon_hooks"] = m
        import antenv
        antenv.axon_hooks = m
        return True
    except Exception:
        return False


def kernel(**inputs):
    global last_exec_time_ns
    from concourse import bass_utils

    nc = _get_nc()
    in_maps = _prep_inputs(**inputs)
    trace = bool(int(os.environ.get("BASS_KERNEL_TRACE", "0")))
    if trace:
        trace = _install_trace_shim()
    res = bass_utils.run_bass_kernel_spmd(
        nc, in_maps, core_ids=list(range(8)), trace=trace
    )
    last_exec_time_ns = res.exec_time_ns
    _CACHE["last_results"] = res
    return np.concatenate(
        [res.results[c]["out"] for c in range(8)], axis=0
    ).astype(np.float32)


# revision 32
# speedup vs baseline: 1.3380x; 1.1120x over previous
"""Trainium2 Bass kernel for the binary CNN (nn_Net_5772436046568).

Data-parallel over 8 NeuronCores: 32 images per core, weights replicated.

Network (per core, B=32):
  conv0 (float 3->128, 3x3 SAME) + BN -> sign      [128, 32x32]
  bconv w1 -> maxpool -> sign                       [128, 16x16]
  bconv w2 -> sign                                  [256, 16x16]
  bconv w3 -> maxpool -> sign                       [256, 8x8]
  bconv w4 -> sign                                  [512, 8x8]
  bconv w5 -> maxpool -> sign                       [512, 4x4]
  fc1 (8192->1024) + BN -> hardtanh -> sign
  fc2 (1024->10) -> log_softmax

All binarized values are in {-1,0,+1} (exact in fp8e4); conv/fc sums are
small integers (exact in fp32 PSUM), so sign() == clip(h,-1,1) exactly.
Convs are 9 shifted fp8 matmuls accumulating in PSUM, reading zero-padded
SBUF activation buffers (channels on partitions, free dim = (b, Hp, Wp)).
"""

import os
import numpy as np
import ml_dtypes

NP8 = ml_dtypes.float8_e4m3
EPS = np.float32(1e-5)
S40 = np.float32(2.0**40)

_CACHE = {}
last_exec_time_ns = None


def _build():
    import concourse.mybir as mybir
    import concourse.tile as tile
    from concourse import bacc

    dt = mybir.dt
    F32 = dt.float32
    FP8 = dt.float8e4
    AL = mybir.AluOpType
    ACT = mybir.ActivationFunctionType

    nc = bacc.Bacc("TRN2", target_bir_lowering=False, debug=False, num_devices=8)

    xcold = nc.dram_tensor("xcol", (16, 128, 512), F32, kind="ExternalInput")
    w0d = nc.dram_tensor("w0", (128, 128), F32, kind="ExternalInput")
    bn0d = nc.dram_tensor("bn0", (128, 2), F32, kind="ExternalInput")
    w1d = nc.dram_tensor("w1", (128, 1152), FP8, kind="ExternalInput")
    w2d = nc.dram_tensor("w2", (128, 2304), FP8, kind="ExternalInput")
    w3d = nc.dram_tensor("w3", (128, 4608), FP8, kind="ExternalInput")
    w4d = nc.dram_tensor("w4", (128, 9216), FP8, kind="ExternalInput")
    w5d = nc.dram_tensor("w5", (128, 18432), FP8, kind="ExternalInput")
    fc1d = nc.dram_tensor("fc1w", (64, 128, 1024), FP8, kind="ExternalInput")
    bnfd = nc.dram_tensor("bnf", (128, 16), F32, kind="ExternalInput")
    fc2d = nc.dram_tensor("fc2w", (128, 80), FP8, kind="ExternalInput")
    outd = nc.dram_tensor("out", (32, 10), F32, kind="ExternalOutput")

    with tile.TileContext(nc) as tc:
        with (
            tc.tile_pool(name="persist", bufs=1) as PP,
            tc.tile_pool(name="tmp", bufs=3) as TP,
        ):
            # ---- persistent weight tiles ----
            w0t = PP.tile([128, 128], F32)
            nc.sync.dma_start(w0t[:], w0d.ap())
            bn0t = PP.tile([128, 2], F32)
            nc.sync.dma_start(bn0t[:], bn0d.ap())
            w1t = PP.tile([128, 1152], FP8)
            nc.sync.dma_start(w1t[:], w1d.ap())
            # tiles for later layers (DMAs emitted after phase A)
            bnft = PP.tile([128, 16], F32)
            fc2t = PP.tile([128, 80], FP8)
            w2t = PP.tile([128, 2304], FP8)
            w3t = PP.tile([128, 2 * 2304], FP8)
            w4t = PP.tile([128, 2 * 4608], FP8)
            w5t = PP.tile([128, 4 * 4608], FP8)

            # ---- persistent activation buffers (padded, fp8) ----
            a2 = PP.tile([128, 32 * 18 * 32], FP8)
            a3 = PP.tile([128, 2 * 32 * 18 * 18], FP8)
            a4 = PP.tile([128, 2 * 32 * 10 * 10], FP8)
            a5 = PP.tile([128, 4 * 32 * 10 * 10], FP8)
            a6 = PP.tile([128, 4 * 32 * 16], FP8)
            a7 = PP.tile([128, 8 * 32], FP8)

            a2v = a2[:].rearrange("c (b h w) -> c b h w", b=32, h=18, w=32)
            a3v = [
                a3[:, kb * 10368 : (kb + 1) * 10368].rearrange(
                    "c (b h w) -> c b h w", b=32, h=18, w=18
                )
                for kb in range(2)
            ]
            a4v = [
                a4[:, kb * 3200 : (kb + 1) * 3200].rearrange(
                    "c (b h w) -> c b h w", b=32, h=10, w=10
                )
                for kb in range(2)
            ]
            a5v = [
                a5[:, kb * 3200 : (kb + 1) * 3200].rearrange(
                    "c (b h w) -> c b h w", b=32, h=10, w=10
                )
                for kb in range(4)
            ]

            # zero padded buffers (borders must be 0 = sign-padding); a1/a2
            # are needed early, the rest is zeroed at phase B start.
            nc.gpsimd.memzero(a2[:, : 16 * 18 * 32])
            nc.gpsimd.memzero(a2[:, 16 * 18 * 32 :])

            # PE warmup: ~8us of dummy matmuls trips the HAM un-throttle
            # (K=4/8 -> 8/8) before the fp32 conv0 stream begins.
            warm = PP.tile([128, 512], FP8)
            nc.vector.memzero(warm[:])

            def clip_store(dst, src):
                nc.vector.tensor_scalar(dst, src, -1.0, 1.0, AL.max, AL.min)

            # ============ phase A: conv0 + conv1 (a1 lives here) ============
            with (
                tc.tile_pool(name="pA", bufs=1) as PA,
                tc.tile_pool(name="xcp", bufs=3) as XC,
                tc.tile_pool(name="psA", bufs=7, space="PSUM") as PSA,
            ):
                a1 = PA.tile([128, 32 * 34 * 48], FP8)
                a1v = a1[:].rearrange("c (b h w) -> c b h w", b=32, h=34, w=48)
                Q = 8 * 34 * 48
                for qi in range(4):
                    nc.gpsimd.memzero(a1[:, qi * Q : (qi + 1) * Q])

                wps = PSA.tile([128, 512], F32, tag="warmps", bufs=1)
                for wi in range(24):
                    nc.tensor.matmul(
                        wps[:], warm[:, :128], warm[:], start=(wi == 0),
                        stop=(wi == 23),
                    )

                # conv0 (fp32, K=27 host im2col) interleaved per-image with
                # conv1 (fp8): the fp8 bursts keep the HAM clock-gate warm --
                # a pure fp32 stream does not register as PE activity and
                # would run the whole phase at 1.2 GHz.
                # conv0 chunks: one row-group = 2 output rows across a
                # 16-image half (host xcol is ordered [half*16+yg, 27, 1024])
                # 4 fp32 K=27 matmuls packed into disjoint 32-row PE
                # strips (tile_position row tiling) run concurrently; each
                # strip gets its own im2col chunk + a replicated w0 block.
                def conv0_tile4(t):
                    xc = XC.tile([128, 512], F32)
                    nc.sync.dma_start(xc[:], xcold.ap()[t])
                    for i in range(4):
                        c = 4 * t + i
                        grp, q = c // 2, c % 2
                        half, yg = grp // 16, grp % 16
                        b0 = 16 * half
                        ps = PSA.tile([128, 512], F32)
                        nc.tensor.matmul(
                            ps[:],
                            w0t[32 * i : 32 * i + 27, :],
                            xc[32 * i : 32 * i + 27, :],
                            start=True, stop=True,
                            tile_position=(32 * i, 0),
                        )
                        t1 = TP.tile([128, 512], F32)
                        # t = (h * inv*2^40) + bias*2^40 on ACT; clip == sign
                        nc.scalar.activation(
                            t1[:], ps[:], ACT.Identity,
                            bias=bn0t[:, 1:2], scale=bn0t[:, 0:1],
                        )
                        clip_store(
                            a1v[:, b0 : b0 + 16, 1 + 2 * yg + q, 1:33], t1[:]
                        )

                # conv1 via dy-pair DoubleRow: pairs shifts (dy=0,dy=1)
                # (a1 row stride 48B is 16-aligned), dy=2 as a normal matmul.
                # Chunk = one output row y over a 16-image half (N=512).
                w1pv = w1t[:].rearrange("c (dy dx n) -> c dy dx n", dy=3, dx=3, n=128)
                DR1 = mybir.MatmulPerfMode.DoubleRow

                def conv1_rows(yp, half):
                    b0 = 16 * half
                    pss = []
                    for par in range(2):
                        y = 2 * yp + par
                        ps_ = PSA.tile(
                            [128, 512], F32, tag="ps", name=f"c1p{yp}{half}{par}"
                        )
                        pss.append(ps_)
                        i = 0
                        for dx in range(3):
                            rhs2 = a1v[:, b0 : b0 + 16, y : y + 2, dx : dx + 32]
                            nc.tensor.matmul(
                                ps_[:],
                                w1pv[:, 0:2, dx, :],
                                rhs2.transpose([0, 2, 1, 3]),
                                start=(i == 0), stop=False,
                                perf_mode=DR1, skip_group_check=True,
                            )
                            i += 1
                        for dx in range(3):
                            nc.tensor.matmul(
                                ps_[:],
                                w1pv[:, 2, dx, :],
                                a1v[:, b0 : b0 + 16, y + 2, dx : dx + 32],
                                start=False, stop=(dx == 2),
                                skip_group_check=True,
                            )
                    te = TP.tile([128, 256], F32)
                    to = TP.tile([128, 256], F32)
                    for ps_, t_ in ((pss[0], te), (pss[1], to)):
                        pvv = ps_[:].rearrange("c (b xp t) -> c b xp t", b=16, xp=16, t=2)
                        hx = TP.tile([128, 256], F32)
                        nc.scalar.copy(hx[:], pvv[:, :, :, 0])
                        nc.vector.tensor_tensor(t_[:], hx[:], pvv[:, :, :, 1], AL.max)
                    p2 = TP.tile([128, 256], F32)
                    nc.vector.tensor_tensor(p2[:], te[:], to[:], AL.max)
                    clip_store(a2v[:, b0 : b0 + 16, 1 + yp, 1:17], p2[:])

                # fine-grained fp32/fp8 interleave: conv1_rows(yp) needs
                # conv0 row-groups 0..yp+1 of its half
                for t in range(16):
                    conv0_tile4(t)
                    k, h = t % 8, t // 8
                    if k >= 1:
                        conv1_rows(2 * k - 2, h)
                        conv1_rows(2 * k - 1, h)
                    elif t == 8:
                        conv1_rows(14, 0)
                        conv1_rows(15, 0)
                conv1_rows(14, 1)
                conv1_rows(15, 1)

            # ============ phase B: conv2..5, fc, softmax ============
            with tc.tile_pool(name="pB", bufs=1) as PB:
                # late weight DMAs + buffer zeroing (overlap with phase A)
                nc.sync.dma_start(w2t[:], w2d.ap())
                nc.sync.dma_start(w3t[:], w3d.ap())
                nc.sync.dma_start(w4t[:], w4d.ap())
                nc.sync.dma_start(w5t[:], w5d.ap())
                nc.sync.dma_start(bnft[:], bnfd.ap())
                nc.sync.dma_start(fc2t[:], fc2d.ap())
                for t in (a3, a4, a5):
                    nc.gpsimd.memzero(t[:])
                fc1t = PB.tile([128, 65536], FP8)
                nc.sync.dma_start(
                    fc1t[:].rearrange("c (a j) -> c a j", a=64),
                    fc1d.ap().rearrange("a c j -> c a j"),
                )

                with tc.tile_pool(name="psB", bufs=8, space="PSUM") as PSB:
                    # conv2: 128->256, 16x16, no pool -> a3 (dy-pair DR)
                    w2pv = w2t[:].rearrange(
                        "c (dy dx n) -> c dy dx n", dy=3, dx=3, n=256
                    )
                    for y in range(16):
                        for mb in range(2):
                            ps = PSB.tile([128, 512], F32, tag="ps", name=f"c2{y}{mb}")
                            i = 0
                            for dx in range(3):
                                rhs2 = a2v[:, :, y : y + 2, dx : dx + 16]
                                nc.tensor.matmul(
                                    ps[:],
                                    w2pv[:, 0:2, dx, mb * 128 : mb * 128 + 128],
                                    rhs2.transpose([0, 2, 1, 3]),
                                    start=(i == 0), stop=False,
                                    perf_mode=mybir.MatmulPerfMode.DoubleRow,
                                    skip_group_check=True,
                                )
                                i += 1
                            for dx in range(3):
                                nc.tensor.matmul(
                                    ps[:],
                                    w2pv[:, 2, dx, mb * 128 : mb * 128 + 128],
                                    a2v[:, :, y + 2, dx : dx + 16],
                                    start=False, stop=(dx == 2),
                                    skip_group_check=True,
                                )
                            clip_store(a3v[mb][:, :, 1 + y, 1:17], ps[:])

                    # conv3: 256->256, 16x16, maxpool -> a4. DoubleRow fp8:
                    # K=256 as channel-block pairs; one output row per psum
                    # (N = 32 images x 16 cols = 512), pool pairs two psums.
                    a3pv = a3[:].rearrange(
                        "c (k b h w) -> c k b h w", k=2, b=32, h=18, w=18
                    )
                    w3pv = w3t[:].rearrange(
                        "c (s mb m two) -> c s mb m two", s=9, mb=2, m=128, two=2
                    )
                    DR = mybir.MatmulPerfMode.DoubleRowSwInterleave
                    for yp in range(8):
                        for mb in range(2):
                            pse = PSB.tile([128, 512], F32, tag="ps", name=f"c3e{yp}{mb}")
                            pso = PSB.tile([128, 512], F32, tag="ps", name=f"c3o{yp}{mb}")
                            for par, ps_ in ((0, pse), (1, pso)):
                                y = 2 * yp + par
                                for s in range(9):
                                    dy, dx = divmod(s, 3)
                                    nc.tensor.matmul(
                                        ps_[:],
                                        w3pv[:, s, mb].transpose([0, 2, 1]),
                                        a3pv[:, :, :, y + dy, dx : dx + 16],
                                        start=(s == 0), stop=(s == 8),
                                        perf_mode=DR,
                                    )
                            te = TP.tile([128, 256], F32)
                            to = TP.tile([128, 256], F32)
                            for ps_, t_ in ((pse, te), (pso, to)):
                                pvv = ps_[:].rearrange(
                                    "c (b xp t) -> c b xp t", b=32, xp=8, t=2
                                )
                                hx = TP.tile([128, 256], F32)
                                nc.scalar.copy(hx[:], pvv[:, :, :, 0])
                                nc.vector.tensor_tensor(
                                    t_[:], hx[:], pvv[:, :, :, 1], AL.max
                                )
                            p2 = TP.tile([128, 256], F32)
                            nc.vector.tensor_tensor(p2[:], te[:], to[:], AL.max)
                            clip_store(a4v[mb][:, :, 1 + yp, 1:9], p2[:])

                    # conv4/conv5 DoubleRow: chunks of 64 consecutive padded
                    # rows R = b*10 + ypad across images (N = 64*8 = 512);
                    # rows with ypad in {8,9} compute garbage that the drain
                    # skips. Row counts clamp at the array end (the clipped
                    # tail rows are garbage rows anyway).
                    def img_segments(R0):
                        # [(b, y0, y1, r_off)] valid interior rows per image
                        segs = []
                        for b in range(R0 // 10, min(32, (R0 + 64 + 9) // 10)):
                            y0 = max(0, R0 - 10 * b)
                            y1 = min(8, R0 + 64 - 10 * b)
                            if y1 > y0:
                                segs.append((b, y0, y1, 10 * b + y0 - R0))
                        return segs

                    # conv4: 256->512, 8x8, no pool -> a5
                    a4pv = a4[:].rearrange("c (k r x) -> c k r x", k=2, r=320, x=10)
                    w4pv = w4t[:].rearrange(
                        "c (s mb m two) -> c s mb m two", s=9, mb=4, m=128, two=2
                    )
                    for ci in range(5):
                        R0 = 64 * ci
                        for mb in range(4):
                            ps = PSB.tile([128, 512], F32, tag="ps", name=f"c4{ci}{mb}")
                            for s in range(9):
                                dy, dx = divmod(s, 3)
                                cnt = min(64, 320 - (R0 + dy))
                                nc.tensor.matmul(
                                    ps[:, : cnt * 8],
                                    w4pv[:, s, mb].transpose([0, 2, 1]),
                                    a4pv[:, :, R0 + dy : R0 + dy + cnt, dx : dx + 8],
                                    start=(s == 0), stop=(s == 8),
                                    perf_mode=DR, skip_group_check=True,
                                )
                            psr = ps[:].rearrange("c (r x) -> c r x", x=8)
                            for b, y0, y1, ro in img_segments(R0):
                                clip_store(
                                    a5v[mb][:, b, 1 + y0 : 1 + y1, 1:9],
                                    psr[:, ro : ro + (y1 - y0), :],
                                )

                    # conv5: 512->512, 8x8, maxpool -> a6 [c, (kb b s)]
                    a5pv = a5[:].rearrange(
                        "c (g k r x) -> c g k r x", g=2, k=2, r=320, x=10
                    )
                    w5pv = w5t[:].rearrange(
                        "c (s mb g m two) -> c s mb g m two", s=9, mb=4, g=2, m=128, two=2
                    )
                    for ci in range(5):
                        R0 = 64 * ci
                        for mb in range(4):
                            ps = PSB.tile([128, 512], F32, tag="ps", name=f"c5{ci}{mb}")
                            i = 0
                            for s in range(9):
                                dy, dx = divmod(s, 3)
                                cnt = min(64, 320 - (R0 + dy))
                                for g in range(2):
                                    nc.tensor.matmul(
                                        ps[:, : cnt * 8],
                                        w5pv[:, s, mb, g].transpose([0, 2, 1]),
                                        a5pv[:, g, :, R0 + dy : R0 + dy + cnt, dx : dx + 8],
                                        start=(i == 0), stop=(i == 17),
                                        perf_mode=DR, skip_group_check=True,
                                    )
                                    i += 1
                            # xmax over col pairs, ymax over row pairs, then
                            # per-image writes of valid pooled rows
                            pvv = ps[:].rearrange("c (r xp t) -> c r xp t", r=64, xp=4, t=2)
                            hx = TP.tile([128, 256], F32)
                            nc.scalar.copy(hx[:], pvv[:, :, :, 0])
                            p1 = TP.tile([128, 256], F32)
                            nc.vector.tensor_tensor(p1[:], hx[:], pvv[:, :, :, 1], AL.max)
                            p1v = p1[:].rearrange("c (r2 t xp) -> c r2 t xp", r2=32, t=2, xp=4)
                            p2 = TP.tile([128, 128], F32)
                            nc.vector.tensor_tensor(
                                p2[:], p1v[:, :, 0, :], p1v[:, :, 1, :], AL.max
                            )
                            # p2 cols: (R0/2 + r2) -> image b = (R0+2*r2)//10,
                            # pooled row m = ((R0+2*r2)%10)//2 valid if %10 < 8
                            for b in range(R0 // 10, min(32, (R0 + 64 + 9) // 10)):
                                m0 = max(0, -(-(R0 - 10 * b) // 2))
                                m1 = min(4, (R0 + 64 - 10 * b) // 2)
                                if m1 > m0:
                                    c0 = (10 * b + 2 * m0 - R0) // 2 * 4
                                    clip_store(
                                        a6[:, mb * 512 + b * 16 + m0 * 4 : mb * 512 + b * 16 + m1 * 4],
                                        p2[:, c0 : c0 + (m1 - m0) * 4],
                                    )

                # fc1: [32, 8192] @ [8192, 1024], j on partitions
                a6v = a6[:].rearrange("c (k b s) -> c k b s", k=4, b=32, s=16)
                with tc.tile_pool(name="psF", bufs=8, space="PSUM") as PSF:
                    psF = [PSF.tile([128, 32], F32, tag="psF", name=f"psF{j}") for j in range(8)]
                    for si in range(16):
                        for kb in range(4):
                            rhs = a6v[:, kb, :, si]
                            for jb in range(8):
                                c0 = (si * 4 + kb) * 1024 + jb * 128
                                nc.tensor.matmul(
                                    psF[jb][:],
                                    fc1t[:, c0 : c0 + 128],
                                    rhs,
                                    start=(si == 0 and kb == 0),
                                    stop=(si == 15 and kb == 3),
                                    skip_group_check=True,
                                )
                    for jb in range(8):
                        t1 = TP.tile([128, 32], F32)
                        nc.vector.tensor_scalar(
                            t1[:], psF[jb][:],
                            bnft[:, jb : jb + 1], bnft[:, 8 + jb : 9 + jb],
                            AL.mult, AL.add,
                        )
                        clip_store(a7[:, jb * 32 : (jb + 1) * 32], t1[:])

                # fc2 + log_softmax
                with tc.tile_pool(name="psZ", bufs=2, space="PSUM") as PSZ:
                    zps = PSZ.tile([32, 10], F32)
                    for jb in range(8):
                        nc.tensor.matmul(
                            zps[:],
                            a7[:, jb * 32 : (jb + 1) * 32],
                            fc2t[:, jb * 10 : (jb + 1) * 10],
                            start=(jb == 0), stop=(jb == 7),
                        )
                    mx = TP.tile([32, 1], F32)
                    nc.vector.pool_max(mx[:], zps[:])
                    ng = TP.tile([32, 1], F32)
                    nc.vector.tensor_scalar(ng[:], mx[:], -1.0, None, AL.mult)
                    ex = TP.tile([32, 10], F32)
                    sm = TP.tile([32, 1], F32)
                    nc.scalar.activation(
                        ex[:], zps[:], ACT.Exp, bias=ng[:], scale=1.0, accum_out=sm[:]
                    )
                    ls = TP.tile([32, 1], F32)
                    nc.scalar.activation(ls[:], sm[:], ACT.Ln)
                    nls = TP.tile([32, 1], F32)
                    nc.vector.tensor_scalar(nls[:], ls[:], -1.0, None, AL.mult)
                    ot = TP.tile([32, 10], F32)
                    nc.vector.tensor_scalar(ot[:], zps[:], ng[:], nls[:], AL.add, AL.add)
                    nc.sync.dma_start(outd.ap(), ot[:])

    nc.finalize()
    return nc


def _get_nc():
    if "nc" not in _CACHE:
        _CACHE["nc"] = _build()
    return _CACHE["nc"]


def _prep_inputs(x, w0, bn1_g, bn1_b, bn1_m, bn1_v, w1, w2, w3, w4, w5,
                 fc1_w, bnf_g, bnf_b, bnf_m, bnf_v, fc2_w):
    f32 = np.float32
    x = np.asarray(x, f32)

    inv1 = (np.asarray(bn1_g, f32) / np.sqrt(np.asarray(bn1_v, f32) + EPS)).astype(f32)
    bias1 = (np.asarray(bn1_b, f32) - np.asarray(bn1_m, f32) * inv1).astype(f32)
    bn0_h = np.stack([inv1 * S40, bias1 * S40], axis=1).astype(f32)  # [128,2]

    invf = (np.asarray(bnf_g, f32) / np.sqrt(np.asarray(bnf_v, f32) + EPS)).astype(f32)
    biasf = (np.asarray(bnf_b, f32) - np.asarray(bnf_m, f32) * invf).astype(f32)
    bnf_h = np.concatenate(
        [(invf * S40).reshape(8, 128).T, (biasf * S40).reshape(8, 128).T], axis=1
    ).astype(f32)  # [128, 16] cols: inv per jb, then bias per jb

    w0_h27 = np.asarray(w0, f32).transpose(2, 3, 1, 0).reshape(27, 128)
    w0_h = np.zeros((128, 128), f32)
    for i in range(4):
        w0_h[32 * i : 32 * i + 27, :] = w0_h27

    def conv_w(w):  # [Cout, Cin, 3, 3] -> sign -> [Cin, 9*Cout] fp8
        w = np.asarray(w, f32)
        cout, cin = w.shape[0], w.shape[1]
        return np.ascontiguousarray(
            np.sign(w).transpose(1, 2, 3, 0).reshape(cin, 9 * cout)
        ).astype(NP8)

    w1_h, w2_h = map(conv_w, (w1, w2))

    def conv_w_swi(w, mbn, gn):
        # [Cout, Cin, 3, 3] -> [128ci, (s, mb, (g,) co_rev, j)] fp8,
        # pairs (A,B)=(kb 2g, 2g+1) interleaved per column, columns reversed
        w = np.sign(np.asarray(w, np.float32))
        t = w.reshape(mbn, 128, gn, 2, 128, 3, 3)  # [mb, co, g, j, ci, dy, dx]
        t = t.transpose(4, 5, 6, 0, 2, 1, 3)       # [ci, dy, dx, mb, g, co, j]
        t = t[:, :, :, :, :, ::-1, :]              # co reversed
        return np.ascontiguousarray(t.reshape(128, -1)).astype(NP8)

    w3_h = conv_w_swi(w3, 2, 1)
    w4_h = conv_w_swi(w4, 4, 1)
    w5_h = conv_w_swi(w5, 4, 2)

    fc1_h = np.ascontiguousarray(
        np.sign(np.asarray(fc1_w, f32))
        .reshape(1024, 512, 16)
        .transpose(2, 1, 0)
        .reshape(64, 128, 1024)
    ).astype(NP8)

    fc2_h = np.ascontiguousarray(
        np.sign(np.asarray(fc2_w, f32)).T.reshape(8, 128, 10).transpose(1, 0, 2).reshape(128, 80)
    ).astype(NP8)

    common = {
        "w0": w0_h, "bn0": bn0_h, "w1": w1_h, "w2": w2_h, "w3": w3_h,
        "w4": w4_h, "w5": w5_h, "fc1w": fc1_h, "bnf": bnf_h, "fc2w": fc2_h,
    }
    in_maps = []
    for c in range(8):
        xs = x[c * 32 : (c + 1) * 32]
        xp = np.zeros((32, 3, 34, 34), f32)
        xp[:, :, 1:33, 1:33] = xs
        # im2col rows (s,ci): [16 chunks, 27, 2 images * 32 * 32]
        win = np.stack(
            [xp[:, :, dy : dy + 32, dx : dx + 32]
             for dy in range(3) for dx in range(3)],
            axis=1,
        )  # [32, 9, 3, 32, 32]
        # reorder to row-groups: [half*16+yg, 27, (ysub2, b16, x32)]
        xcol = (
            win.reshape(2, 16, 9, 3, 32, 32)   # [half, b, s, ci, y, x]
            .transpose(0, 4, 2, 3, 1, 5)       # [half, y, s, ci, b, x]
            .reshape(2, 16, 2, 27, 16, 32)     # [half, yg, ysub, sci, b, x]
            .transpose(0, 1, 3, 2, 4, 5)       # [half, yg, sci, ysub, b, x]
            .reshape(32, 27, 1024)
        )
        # pack 4 consecutive 512-col chunks into the 4 PE row strips
        chunks = xcol.reshape(32, 27, 2, 512).transpose(0, 2, 1, 3).reshape(64, 27, 512)
        xc4 = np.zeros((16, 128, 512), f32)
        for i in range(4):
            xc4[:, 32 * i : 32 * i + 27, :] = chunks[i::4]
        xcol = np.ascontiguousarray(xc4)
        in_maps.append({**common, "xcol": xcol})
    return in_maps


def _install_trace_shim():
    """The image's antenv lacks axon_hooks; recreate it from trn_agent_boot
    so run_bass_kernel_spmd(trace=True) can capture NTFF profiles."""
    import sys, types
    if "antenv.axon_hooks" in sys.modules:
        return True
    try:
        from trn_agent_boot.trn_boot import _ntff_profile_via_ctypes
        h = _ntff_profile_via_ctypes("/opt/axon/libaxon_pjrt.so")
        if h is None:
            return False
        m = types.ModuleType("antenv.axon_hooks")
        m.get_axon_ntff_profile_hook = lambda: h
        m.set_axon_ntff_profile_hook = lambda hook: None
        sys.modules["antenv.axon_hooks"] = m
        import antenv
        antenv.axon_hooks = m
        return True
    except Exception:
        return False


def kernel(**inputs):
    global last_exec_time_ns
    from concourse import bass_utils

    nc = _get_nc()
    in_maps = _prep_inputs(**inputs)
    trace = bool(int(os.environ.get("BASS_KERNEL_TRACE", "0")))
    if trace:
        trace = _install_trace_shim()
    res = bass_utils.run_bass_kernel_spmd(
        nc, in_maps, core_ids=list(range(8)), trace=trace
    )
    last_exec_time_ns = res.exec_time_ns
    _CACHE["last_results"] = res
    return np.concatenate(
        [res.results[c]["out"] for c in range(8)], axis=0
    ).astype(np.float32)
